# revision 1
# baseline (speedup 1.0000x reference)
"""Trainium2 Bass kernel: 8 independent 3x3 filters applied to every channel.

Reference op: x[B=8, C=32, 224, 224], W[1, 8, 3, 3], Bv[8]
  -> y[B, 8*C, 222, 222],  y[b, d*C+c, i, j] = sum_{u,v} x[b,c,i+u,j+v] W[0,d,u,v] + Bv[d]

Sharding: data-parallel over batch B across the 8 cores (core k takes x[k]).

Per-core formulation (all compute on TensorE):
  Matmul M-columns are (d', rl) = 4 filters x 28 row-groups; each PSUM
  partition accumulates 4 consecutive output rows (r = base + 4*rl + t) via
  4 sub-row matmul groups t writing different PSUM column ranges:
    psum[(d'*28+rl), (img, t, j)] += sum_r LW[r, ...] * TILE[r, img, j+v]
  with LW[local(base+4rl+t)+u, ..., d'*28+rl] = W[0, 4dh+d', u, v] a banded
  weight matrix (band truncated at K=128; spill rows land in the 2 pad rows).
  K = 128 (full input tile on partitions, base 0), N = 444 = 2 images x 222
  (N>=256 keeps float32r matmuls at 1 cycle/row). The 3 v-shift matmuls
  accumulate in PSUM; bias is added during the PSUM->SBUF copy (DVE
  tensor_scalar). Each partition's accumulated (sb, t, j) block is fully
  contiguous in the permuted DRAM layout => 7104B DMA descriptors and one
  fully-contiguous 199KB DMA per output channel (host un-permutes rows).

Super-blocks: sb0 = output rows 0..111 from input tile rows 0:128,
sb1 = output rows 112..223 from input tile rows 96:224 (rows 222/223 are
garbage from band truncation and land in the 2 DRAM pad rows per channel).
"""

import os
import numpy as np

B, C, H, W_IN = 8, 32, 224, 224
ND, KS = 8, 3
HO, WO = 222, 222
NCORES = 8
GSZ = 8        # images per input-tile group
NRL = 28       # row-groups per super-block
NT = 4         # rows per row-group
MM = 4 * NRL   # matmul M (112)
TILE_ROWS = [(0, 128), (96, 128)]   # (dram row base, K)
SB_BASE = [0, 112]                  # output row base per super-block

_PROG_CACHE = {}


def _build(mode: str, n_imgs: int):
    """Build+compile the per-core Bass program.

    mode: 'f32' (exact, 4 cyc/row), 'f32r' (relaxed fp32, 1 cyc/row @ N>=256),
          'bf16' (host-cast inputs).
    """
    import concourse.mybir as mybir
    import concourse.tile as tile
    from concourse import bacc

    dt = mybir.dt
    if mode == "bf16":
        io_dt = dt.bfloat16
    elif mode == "f32r":
        io_dt = dt.float32r
    else:
        io_dt = dt.float32

    n_groups = n_imgs // GSZ
    assert n_imgs % GSZ == 0

    nc = bacc.Bacc("TRN2", target_bir_lowering=False, debug=False)
    xin = nc.dram_tensor("xin", [n_imgs, H, W_IN], io_dt, kind="ExternalInput")
    lw = nc.dram_tensor("lw", [128, 2, NT, 3, 2, MM], io_dt,
                        kind="ExternalInput")
    bias = nc.dram_tensor("bias", [MM, 2], dt.float32, kind="ExternalInput")
    # permuted row order (rl, sb, t): row r = 112*sb + 4*rl + t lives at
    # [rl, sb, t]; host un-permutes. Garbage rows 222/223 are (rl=27, sb=1,
    # t=2/3) and are dropped on the host. This makes each (img, dh) output a
    # single fully-contiguous-per-channel 796KB DMA with 7104B descriptors.
    # image-major so each (img, dh) DMA's 4 channels are DRAM-adjacent:
    # the DMA's DRAM AP merges to 2 dims (3-dim APs run HWDGE descriptor
    # generation ~3x slower: 102 vs 276 GB/s measured).
    # pair-interleaved: [pair, ch, rl, sb, t, img, j] so one DMA per
    # (pair, dh) covers 4 channels x 2 images = 1.59MB, fully merging to a
    # 1-dim DRAM AP with 14.2KB descriptors.
    yout = nc.dram_tensor("yout", [n_imgs // 2, ND, NRL, 2, NT, 2, WO],
                          dt.float32, kind="ExternalOutput")

    with tile.TileContext(nc) as tc:
        with (
            tc.tile_pool(name="const", bufs=1) as constp,
            tc.tile_pool(name="inp", bufs=3) as inp,
            tc.tile_pool(name="outp", bufs=3) as outp,
            tc.tile_pool(name="psum", bufs=8, space="PSUM") as psp,
        ):
            # per-(sb,t) weight tiles: first matmul gates on one 344KB DMA
            # (a single lw tile made it wait for the whole 2.75MB constant);
            # all 8 loads still emitted upfront, split across both rings in
            # the order pair 0 consumes them
            lwt = [[constp.tile([128, 3, 2, MM], io_dt, name=f"lw{s}{tt}")
                    for tt in range(NT)] for s in range(2)]
            for i, (s, tt) in enumerate(
                    [(s, tt) for s in range(2) for tt in range(NT)]):
                leng = nc.sync if i % 2 == 0 else nc.scalar
                leng.dma_start(lwt[s][tt][:], lw[:, s, tt, :, :, :])
            bias_sb = constp.tile([MM, 2], dt.float32)
            nc.scalar.dma_start(bias_sb[:], bias[:])

            def load_group(g):
                g8 = g * GSZ
                tiles = []
                for ti, (r0, nr) in enumerate(TILE_ROWS):
                    t = inp.tile([nr, GSZ, W_IN], io_dt, name=f"t{ti}",
                                 tag=f"t{ti}")
                    if ti == 0:
                        # per-image 2-dim DMAs on the HWDGE rings
                        for im in range(GSZ):
                            ieng = nc.sync if im % 2 == 0 else nc.scalar
                            ieng.dma_start(t[:, im, :],
                                           xin[g8 + im, r0:r0 + nr, :])
                    else:
                        # batched 3-dim load on the idle SWDGE queue; its
                        # slower descriptor-gen hides in the prefetch lead
                        nc.gpsimd.dma_start(
                            t[:],
                            xin[g8:g8 + GSZ, r0:r0 + nr, :].transpose([1, 0, 2]))
                    tiles.append(t)
                return tiles

            next_tiles = load_group(0)
            for g in range(n_groups):
                g8 = g * GSZ
                tiles = next_tiles
                for pr in range(GSZ // 2):
                    if pr == 1 and g + 1 < n_groups:
                        next_tiles = load_group(g + 1)
                    # acc[dh]: [112, img, sb, t, j]; per partition per image
                    # the (sb, t, j) block maps to 2x 4-consecutive-DRAM-rows
                    # acc[dh]: [112, sb, t, img, j]; per-partition free run
                    # (sb, t, img, j) = 3552 elems contiguous in DRAM
                    acc = [
                        outp.tile([MM, 2, NT, 2, WO], dt.float32,
                                  name=f"acc{dh}", tag=f"acc{dh}")
                        for dh in range(2)
                    ]
                    for dh in range(2):
                        for sb in range(2):
                            src = tiles[sb]
                            for tt in range(NT):
                                ps = psp.tile([MM, 2, WO], dt.float32,
                                              name="ps")
                                for v in range(3):
                                    nc.tensor.matmul(
                                        ps[:],
                                        lwt[sb][tt][:, v, dh, :],
                                        src[:, 2 * pr:2 * pr + 2, v:v + WO],
                                        start=(v == 0),
                                        stop=(v == 2),
                                    )
                                nc.vector.tensor_scalar_add(
                                    acc[dh][:, sb, tt, :, :],
                                    ps[:],
                                    bias_sb[:, dh:dh + 1],
                                )
                        # acc[dh] complete: one 1.59MB DMA for the pair
                        # (last pair: split into channel-pair halves across
                        # both rings to halve the final drain)
                        pair = g * (GSZ // 2) + pr
                        if pair == n_imgs // 2 - 1:
                            for hh in range(2):
                                heng = nc.sync if (dh + hh) % 2 == 0 \
                                    else nc.scalar
                                heng.dma_start(
                                    yout[pair,
                                         4 * dh + 2 * hh:4 * dh + 2 * hh + 2,
                                         :, :, :, :, :],
                                    acc[dh][56 * hh:56 * hh + 56])
                        else:
                            eng = (nc.sync if (pair + dh) % 2 == 0
                                   else nc.scalar)
                            eng.dma_start(
                                yout[pair, 4 * dh:4 * dh + 4, :, :, :, :, :],
                                acc[dh][:])

    nc.compile()
    return nc


def _get_prog(mode: str, n_imgs: int = C):
    key = (mode, n_imgs)
    if key not in _PROG_CACHE:
        _PROG_CACHE[key] = _build(mode, n_imgs)
    return _PROG_CACHE[key]


def _host_weights(W: np.ndarray, Bv: np.ndarray, mode: str):
    """LW[lr, sb, t, v, dh, d'*28+rl] = W[0, 4dh+d', u, v] where
    lr = (SB_BASE[sb] + 4*rl + t + u) - TILE_ROWS[sb][0], clipped to <128.
    bias[d'*28+rl, dh] = Bv[4dh+d']."""
    W = np.asarray(W, np.float32)
    LW = np.zeros((128, 2, NT, 3, 2, MM), np.float32)
    for sb in range(2):
        tile_base = TILE_ROWS[sb][0]
        out_base = SB_BASE[sb]
        for tt in range(NT):
            for v in range(3):
                for dh in range(2):
                    for dd in range(4):
                        for rl in range(NRL):
                            for u in range(3):
                                lr = out_base + 4 * rl + tt + u - tile_base
                                if 0 <= lr < 128:
                                    LW[lr, sb, tt, v, dh, dd * NRL + rl] = \
                                        W[0, 4 * dh + dd, u, v]
    bias = np.stack(
        [np.repeat(np.asarray(Bv[4 * dh:4 * dh + 4], np.float32), NRL)
         for dh in range(2)], axis=1)
    if mode == "bf16":
        import ml_dtypes
        LW = LW.astype(ml_dtypes.bfloat16)
    return np.ascontiguousarray(LW), np.ascontiguousarray(bias)


def _cast_in(x: np.ndarray, mode: str):
    if mode == "bf16":
        import ml_dtypes
        return np.ascontiguousarray(x).astype(ml_dtypes.bfloat16)
    return np.ascontiguousarray(x, np.float32)


def kernel(x, W, Bv, mode: str | None = None, _trace: bool = False):
    from concourse.bass_utils import run_bass_kernel_spmd

    mode = mode or os.environ.get("DCONV_MODE", "f32r")
    x = np.asarray(x, np.float32)
    W = np.asarray(W, np.float32)
    Bv = np.asarray(Bv, np.float32)

    nc = _get_prog(mode)
    LW, bias = _host_weights(W, Bv, mode)
    in_maps = [
        {"xin": _cast_in(x[k], mode), "lw": LW, "bias": bias}
        for k in range(NCORES)
    ]
    res = run_bass_kernel_spmd(nc, in_maps, core_ids=list(range(NCORES)),
                               trace=_trace)
    # yout is [pair, ch, rl, sb, t, img, j]; reorder to (d, pair, img) =
    # channels, (sb, rl, t) = row-major rows, drop the 2 pad rows.
    y = np.stack(
        [np.ascontiguousarray(
            np.asarray(res.results[k]["yout"]).transpose(1, 0, 5, 3, 2, 4, 6)
            .reshape(ND * C, 224, WO)[:, :HO, :]
        ) for k in range(NCORES)],
        axis=0,
    )
    if _trace:
        return y, res
    return y



# revision 2
# speedup vs baseline: 1.0736x; 1.0736x over previous
"""Trainium2 Bass kernel v2: 8 independent 3x3 filters on every channel.

Reference op: x[B=8, C=32, 224, 224], W[1, 8, 3, 3], Bv[8]
  -> y[B, 8*C, 222, 222],  y[b, d*C+c, i, j] = sum_{u,v} x[b,c,i+u,j+v] W[0,d,u,v] + Bv[d]

Sharding: data-parallel over batch B across the 8 cores (core k takes x[k]).

v2 design (vs baseline): fold BOTH conv taps into the stationary band so
every output element is produced by exactly ONE matmul:
  moving tile VT[32*v + r, c, j] = x[c, r0+r, j+v]   (K = 3*32 = 96)
  stationary LW[32*v + r, (d, ro)] = W[d, r-i, v]    (banded; i = out row)
  psum[(d, ro), (c, j)] = finished conv row => 1 matmul per output element
  (vs 3 accumulating matmuls in the baseline), M=128 (vs 112).
v-regions sit at partition bases 0/32/64 (engine APs require mod-32
partition bases). Row-tiles advance 30 output rows (mh0: ro 0..15 M=128,
mh1: ro 16..29 M=112); 8 tiles cover 224 rows (tail rows are dropped pad).
The two v-shift copies are built on-chip from the v0 region: v1 (+1 elem,
bf16) split across Act+DVE, v2 (+2 elems = +1 f32, bitcast-packed). DMA
loads only the raw rows (host pre-permutes x to [r, c, j] so descriptors
are 14336B). PSUM f32 drains to bf16 SBUF in 4-bank groups with the bias
added (tensor_scalar_add on DVE / activation-Identity on Act, alternating)
and ships as fully-contiguous ~1.8 MB DMAs per (tile, mh). Output is bf16
(adds ~2e-3 rel err vs the 2e-2 gate); host casts/unpermutes.
"""

import os
import numpy as np

B, C, H, W_IN = 8, 32, 224, 224
ND, KS = 8, 3
HO, WO = 222, 222
NCORES = 8
RT = 32          # rows per v-region (= input rows loaded per tile)
STRIDE = 30      # output rows advanced per row-tile
NT = 8           # row-tiles (7*30 + 16 >= 224)
K = 3 * RT       # matmul contraction
M1 = 112         # mh1 columns: 8 filters x 14 rows
NCP = C // 2     # channel-pairs (N = 2*222 = 444)
GRP = 2          # psum banks (matmuls) per drain group

_PROG_CACHE = {}


def _build(zero_bias: bool):
    import concourse.mybir as mybir
    import concourse.tile as tile
    from concourse import bacc

    dt = mybir.dt
    bf = dt.bfloat16

    nc = bacc.Bacc("TRN2", target_bir_lowering=False, debug=False)
    # host pre-permuted input [r, c, j]: each partition-row's (c, j) free
    # block is one contiguous 14336B DRAM run
    xin = nc.dram_tensor("xin", [H, C, W_IN], bf, kind="ExternalInput")
    lw = nc.dram_tensor("lw", [K, 2, 128], bf, kind="ExternalInput")
    bias = nc.dram_tensor("bias", [128, 2], dt.float32, kind="ExternalInput")
    # [tile, mh, (d, ro), cp, c2, j]; mh1 uses only 112 partitions; host
    # un-permutes and drops pad rows
    yout = nc.dram_tensor("yout", [NT, 2, 128, NCP, 2, WO], bf,
                          kind="ExternalOutput")

    with tile.TileContext(nc) as tc:
        with (
            tc.tile_pool(name="const", bufs=1) as constp,
            tc.tile_pool(name="inp", bufs=3) as inp,
            tc.tile_pool(name="outp", bufs=4) as outp,
            tc.tile_pool(name="psum", bufs=4, space="PSUM") as psp,
        ):
            lwt = constp.tile([K, 2, 128], bf)
            nc.scalar.dma_start(lwt[:], lw[:])
            bias_sb = constp.tile([128, 2], dt.float32)
            nc.scalar.dma_start(bias_sb[:], bias[:])

            def start_load(t):
                # SWDGE queue: keeps both HWDGE-capable queues (sync for
                # outputs, scalar for Act compute) free of input triggers
                r0 = STRIDE * t
                nr = min(RT, H - r0)
                vt = inp.tile([K, C, W_IN], bf, name="vt", tag="vt")
                nc.gpsimd.dma_start(vt[0:nr, :, :], xin[r0:r0 + nr, :, :])
                return vt

            def do_copies(vt):
                # all on DVE: its copies hit the 2-port fast path (~3x Act)
                nc.vector.tensor_copy(vt[RT:2 * RT, :, 0:W_IN - 1],
                                      vt[0:RT, :, 1:W_IN])
                nc.vector.tensor_copy(
                    vt[2 * RT:3 * RT, :, 0:W_IN - 2].bitcast(dt.float32),
                    vt[0:RT, :, 2:W_IN].bitcast(dt.float32))

            def drain(gi, out_ap, in_ap, bias_ap):
                # Act takes 2/3 of drains (DVE also carries the v-copies);
                # with zero bias use plain dtype-cast copies
                if gi % 3 == 2:
                    if zero_bias:
                        nc.vector.tensor_copy(out_ap, in_ap)
                    else:
                        nc.vector.tensor_scalar_add(out_ap, in_ap, bias_ap)
                else:
                    if zero_bias:
                        nc.scalar.copy(out_ap, in_ap)
                    else:
                        nc.scalar.add(out_ap, in_ap, bias_ap)

            vt = start_load(0)
            do_copies(vt)
            gi = 0
            di = [0]
            for t in range(NT):
                cur = vt
                if t + 1 < NT:
                    vt = start_load(t + 1)
                for mh in range(2):
                    if t == NT - 1 and mh == 1:
                        break  # tail tile: rows 226+ don't exist
                    mm = 128 if mh == 0 else M1
                    acc = outp.tile([mm, NCP, 2, WO], bf, name="acc",
                                    tag="acc")
                    ng = NCP // GRP
                    for g in range(ng):
                        ps = psp.tile([128, GRP, 512], dt.float32, name="ps")
                        for q in range(GRP):
                            cp = g * GRP + q
                            nc.tensor.matmul(
                                ps[0:mm, q, 0:2 * WO],
                                lwt[:, mh, 0:mm],
                                cur[:, 2 * cp:2 * cp + 2, 0:WO],
                                start=True, stop=True)
                        drain(gi, acc[:, g * GRP:(g + 1) * GRP, :, :],
                              ps[0:mm, :, 0:2 * WO], bias_sb[0:mm, mh:mh + 1])
                        gi += 1
                    if mh == 0 and t + 1 < NT:
                        # next tile's shift-copies sit in the DVE queue after
                        # mh0's drains, with a full mh of DMA lead time
                        do_copies(vt)
                    # all output DMAs on the sync queue: a dma_start trigger
                    # waiting on drain sems would stall the Act engine's
                    # in-order compute stream if issued on scalar
                    if t == NT - 1:
                        # tail: quarters, last one on the (now idle) scalar
                        # ring to halve the epilogue drain
                        nc.sync.dma_start(yout[t, mh, 0:mm, 0:8],
                                          acc[:, 0:8])
                        nc.sync.dma_start(yout[t, mh, 0:mm // 2, 8:16],
                                          acc[0:mm // 2, 8:16])
                        nc.scalar.dma_start(yout[t, mh, mm // 2:mm, 8:16],
                                            acc[mm // 2:mm, 8:16])
                    else:
                        nc.sync.dma_start(yout[t, mh, 0:mm], acc[:])

    nc.compile()
    return nc


def _get_prog(zero_bias: bool):
    key = ("v2", zero_bias)
    if key not in _PROG_CACHE:
        _PROG_CACHE[key] = _build(zero_bias)
    return _PROG_CACHE[key]


def _host_weights(W: np.ndarray, Bv: np.ndarray):
    """LW[32*v + r, mh, col]: mh0 col=(d,ro) d*16+ro, out row i=ro;
    mh1 col=d*14+ro', i=16+ro'. Band: W[d, r-i, v] for 0 <= r-i <= 2.
    bias[col, mh] = Bv[d]."""
    import ml_dtypes
    W = np.asarray(W, np.float32)
    Bv = np.asarray(Bv, np.float32)
    LW = np.zeros((K, 2, 128), np.float32)
    bias = np.zeros((128, 2), np.float32)
    for d in range(ND):
        for v in range(3):
            for u in range(3):
                for ro in range(16):
                    LW[RT * v + ro + u, 0, d * 16 + ro] = W[0, d, u, v]
                for ro in range(14):
                    LW[RT * v + 16 + ro + u, 1, d * 14 + ro] = W[0, d, u, v]
        bias[d * 16:d * 16 + 16, 0] = Bv[d]
        bias[d * 14:d * 14 + 14, 1] = Bv[d]
    return (np.ascontiguousarray(LW).astype(ml_dtypes.bfloat16),
            np.ascontiguousarray(bias))


def kernel(x, W, Bv, mode=None, _trace: bool = False):
    import ml_dtypes
    from concourse.bass_utils import run_bass_kernel_spmd

    x = np.asarray(x, np.float32)
    zero_bias = bool(np.all(np.asarray(Bv, np.float32) == 0.0))
    nc = _get_prog(zero_bias)
    LW, bias = _host_weights(W, Bv)
    # per-core input: [c, r, j] -> [r, c, j], bf16
    xp = np.ascontiguousarray(x.transpose(0, 2, 1, 3)).astype(
        ml_dtypes.bfloat16)
    in_maps = [{"xin": xp[k], "lw": LW, "bias": bias} for k in range(NCORES)]
    res = run_bass_kernel_spmd(nc, in_maps, core_ids=list(range(NCORES)),
                               trace=_trace)
    ys = []
    for k in range(NCORES):
        yr = np.asarray(res.results[k]["yout"]).astype(np.float32)
        ybuf = np.empty((ND, C, 240, WO), np.float32)
        for t in range(NT):
            i0 = STRIDE * t
            # mh0: [128, cp, c2, j] -> [d, ro, c, j]
            b0 = yr[t, 0].reshape(ND, 16, NCP, 2, WO)
            ybuf[:, :, i0:i0 + 16] = b0.reshape(
                ND, 16, C, WO).transpose(0, 2, 1, 3)
            if t < NT - 1:
                b1 = yr[t, 1, 0:M1].reshape(ND, 14, NCP, 2, WO)
                ybuf[:, :, i0 + 16:i0 + 30] = b1.reshape(
                    ND, 14, C, WO).transpose(0, 2, 1, 3)
        ys.append(ybuf[:, :, :HO, :].reshape(ND * C, HO, WO))
    y = np.stack(ys, axis=0)
    if _trace:
        return y, res
    return y


# revision 3
# speedup vs baseline: 1.1347x; 1.0569x over previous
"""Trainium2 Bass kernel v2: 8 independent 3x3 filters on every channel.

Reference op: x[B=8, C=32, 224, 224], W[1, 8, 3, 3], Bv[8]
  -> y[B, 8*C, 222, 222],  y[b, d*C+c, i, j] = sum_{u,v} x[b,c,i+u,j+v] W[0,d,u,v] + Bv[d]

Sharding: data-parallel over batch B across the 8 cores (core k takes x[k]).

v2 design (vs baseline): fold BOTH conv taps into the stationary band so
every output element is produced by exactly ONE matmul:
  moving tile VT[32*v + r, c, j] = x[c, r0+r, j+v]   (K = 3*32 = 96)
  stationary LW[32*v + r, (d, ro)] = W[d, r-i, v]    (banded; i = out row)
  psum[(d, ro), (c, j)] = finished conv row => 1 matmul per output element
  (vs 3 accumulating matmuls in the baseline), M=128 (vs 112).
v-regions sit at partition bases 0/32/64 (engine APs require mod-32
partition bases). Row-tiles advance 30 output rows (mh0: ro 0..15 M=128,
mh1: ro 16..29 M=112); 8 tiles cover 224 rows (tail rows are dropped pad).
The two v-shift copies are built on-chip from the v0 region: v1 (+1 elem,
bf16) split across Act+DVE, v2 (+2 elems = +1 f32, bitcast-packed). DMA
loads only the raw rows (host pre-permutes x to [r, c, j] so descriptors
are 14336B). PSUM f32 drains to bf16 SBUF in 4-bank groups with the bias
added (tensor_scalar_add on DVE / activation-Identity on Act, alternating)
and ships as fully-contiguous ~1.8 MB DMAs per (tile, mh). Output is bf16
(adds ~2e-3 rel err vs the 2e-2 gate); host casts/unpermutes.
"""

import os
import numpy as np

B, C, H, W_IN = 8, 32, 224, 224
ND, KS = 8, 3
HO, WO = 222, 222
NCORES = 8
RT = 32          # rows per v-region (= input rows loaded per tile)
STRIDE = 30      # output rows advanced per row-tile
NT = 8           # row-tiles (7*30 + 16 >= 224)
K = 3 * RT       # matmul contraction
M1 = 112         # mh1 columns: 8 filters x 14 rows
NCP = C // 2     # channel-pairs (N = 2*222 = 444)
GRP = 2          # psum banks (matmuls) per drain group

_PROG_CACHE = {}


def _build(zero_bias: bool):
    import concourse.mybir as mybir
    import concourse.tile as tile
    from concourse import bacc

    dt = mybir.dt
    bf = dt.bfloat16

    nc = bacc.Bacc("TRN2", target_bir_lowering=False, debug=False)
    # host pre-permuted input [r, c, j]: each partition-row's (c, j) free
    # block is one contiguous 14336B DRAM run
    xin = nc.dram_tensor("xin", [H, C, W_IN], bf, kind="ExternalInput")
    lw = nc.dram_tensor("lw", [K, 3, 128], bf, kind="ExternalInput")
    bias = nc.dram_tensor("bias", [128, 3], dt.float32, kind="ExternalInput")
    # [tile, mh, (d, ro), cp, c2, j]; mh1 uses only 112 partitions; host
    # un-permutes and drops pad rows
    yout = nc.dram_tensor("yout", [NT, 2, 128, NCP, 2, WO], bf,
                          kind="ExternalOutput")

    with tile.TileContext(nc) as tc:
        with (
            tc.tile_pool(name="const", bufs=1) as constp,
            tc.tile_pool(name="inp", bufs=3) as inp,
            tc.tile_pool(name="outp", bufs=4) as outp,
            tc.tile_pool(name="psum", bufs=4, space="PSUM") as psp,
        ):
            lwt = constp.tile([K, 3, 128], bf)
            nc.scalar.dma_start(lwt[:], lw[:])
            bias_sb = constp.tile([128, 3], dt.float32)
            nc.scalar.dma_start(bias_sb[:], bias[:])

            def start_load(t):
                # SWDGE queue: keeps both HWDGE-capable queues (sync for
                # outputs, scalar for Act compute) free of input triggers
                r0 = STRIDE * t
                nr = min(RT, H - r0)
                vt = inp.tile([K, C, W_IN], bf, name="vt", tag="vt")
                nc.gpsimd.dma_start(vt[0:nr, :, :], xin[r0:r0 + nr, :, :])
                return vt

            def do_copies(vt):
                # all on DVE: its copies hit the 2-port fast path (~3x Act)
                nc.vector.tensor_copy(vt[RT:2 * RT, :, 0:W_IN - 1],
                                      vt[0:RT, :, 1:W_IN])
                nc.vector.tensor_copy(
                    vt[2 * RT:3 * RT, :, 0:W_IN - 2].bitcast(dt.float32),
                    vt[0:RT, :, 2:W_IN].bitcast(dt.float32))

            def drain(gi, out_ap, in_ap, bias_ap):
                # Act takes 2/3 of drains (DVE also carries the v-copies);
                # with zero bias use plain dtype-cast copies
                if gi % 3 == 2:
                    if zero_bias:
                        nc.vector.tensor_copy(out_ap, in_ap)
                    else:
                        nc.vector.tensor_scalar_add(out_ap, in_ap, bias_ap)
                else:
                    if zero_bias:
                        nc.scalar.copy(out_ap, in_ap)
                    else:
                        nc.scalar.add(out_ap, in_ap, bias_ap)

            vt = start_load(0)
            do_copies(vt)
            gi = 0
            di = [0]
            for t in range(NT):
                cur = vt
                if t + 1 < NT:
                    vt = start_load(t + 1)
                for mh in range(2):
                    if t == NT - 1 and mh == 1:
                        break  # tail tile: rows 226+ don't exist
                    # tail tile only has 12 real out rows -> dedicated M=96
                    # column layout so no pad rows are drained or shipped
                    li = 2 if t == NT - 1 else mh
                    mm = (128, M1, 96)[li]
                    acc = outp.tile([mm, NCP, 2, WO], bf, name="acc",
                                    tag="acc")
                    ng = NCP // GRP
                    for g in range(ng):
                        ps = psp.tile([128, GRP, 512], dt.float32, name="ps")
                        for q in range(GRP):
                            cp = g * GRP + q
                            nc.tensor.matmul(
                                ps[0:mm, q, 0:2 * WO],
                                lwt[:, li, 0:mm],
                                cur[:, 2 * cp:2 * cp + 2, 0:WO],
                                start=True, stop=True)
                        drain(gi, acc[:, g * GRP:(g + 1) * GRP, :, :],
                              ps[0:mm, :, 0:2 * WO], bias_sb[0:mm, li:li + 1])
                        gi += 1
                        # first tile: ship half-accs as soon as their drains
                        # land so the output stream starts ~4us earlier
                        if t == 0 and g in (3, 7):
                            cp1 = (g + 1) * GRP
                            nc.sync.dma_start(
                                yout[t, mh, 0:mm, cp1 - 8:cp1],
                                acc[:, cp1 - 8:cp1])
                    if mh == 0 and t + 1 < NT:
                        # next tile's shift-copies sit in the DVE queue after
                        # mh0's drains, with a full mh of DMA lead time
                        do_copies(vt)
                    # all output DMAs on the sync queue: a dma_start trigger
                    # waiting on drain sems would stall the Act engine's
                    # in-order compute stream if issued on scalar
                    if t == NT - 1:
                        # tail: quarters, last one on the (now idle) scalar
                        # ring to halve the epilogue drain
                        nc.sync.dma_start(yout[t, mh, 0:mm, 0:8],
                                          acc[:, 0:8])
                        nc.sync.dma_start(yout[t, mh, 0:mm // 2, 8:16],
                                          acc[0:mm // 2, 8:16])
                        nc.scalar.dma_start(yout[t, mh, mm // 2:mm, 8:16],
                                            acc[mm // 2:mm, 8:16])
                    elif t > 0:
                        nc.sync.dma_start(yout[t, mh, 0:mm], acc[:])

    nc.compile()
    return nc


def _get_prog(zero_bias: bool):
    key = ("v2", zero_bias)
    if key not in _PROG_CACHE:
        _PROG_CACHE[key] = _build(zero_bias)
    return _PROG_CACHE[key]


def _host_weights(W: np.ndarray, Bv: np.ndarray):
    """LW[32*v + r, li, col] for 3 layouts: li=0 col=d*16+ro (i=ro),
    li=1 col=d*14+ro (i=16+ro), li=2 col=d*12+ro (i=ro, tail tile).
    Band: W[d, r-i, v] for 0 <= r-i <= 2. bias[col, li] = Bv[d]."""
    import ml_dtypes
    W = np.asarray(W, np.float32)
    Bv = np.asarray(Bv, np.float32)
    LW = np.zeros((K, 3, 128), np.float32)
    bias = np.zeros((128, 3), np.float32)
    for d in range(ND):
        for v in range(3):
            for u in range(3):
                for ro in range(16):
                    LW[RT * v + ro + u, 0, d * 16 + ro] = W[0, d, u, v]
                for ro in range(14):
                    LW[RT * v + 16 + ro + u, 1, d * 14 + ro] = W[0, d, u, v]
                for ro in range(12):
                    LW[RT * v + ro + u, 2, d * 12 + ro] = W[0, d, u, v]
        bias[d * 16:d * 16 + 16, 0] = Bv[d]
        bias[d * 14:d * 14 + 14, 1] = Bv[d]
        bias[d * 12:d * 12 + 12, 2] = Bv[d]
    return (np.ascontiguousarray(LW).astype(ml_dtypes.bfloat16),
            np.ascontiguousarray(bias))


def kernel(x, W, Bv, mode=None, _trace: bool = False):
    import ml_dtypes
    from concourse.bass_utils import run_bass_kernel_spmd

    x = np.asarray(x, np.float32)
    zero_bias = bool(np.all(np.asarray(Bv, np.float32) == 0.0))
    nc = _get_prog(zero_bias)
    LW, bias = _host_weights(W, Bv)
    # per-core input: [c, r, j] -> [r, c, j], bf16
    xp = np.ascontiguousarray(x.transpose(0, 2, 1, 3)).astype(
        ml_dtypes.bfloat16)
    in_maps = [{"xin": xp[k], "lw": LW, "bias": bias} for k in range(NCORES)]
    res = run_bass_kernel_spmd(nc, in_maps, core_ids=list(range(NCORES)),
                               trace=_trace)
    ys = []
    for k in range(NCORES):
        yr = np.asarray(res.results[k]["yout"]).astype(np.float32)
        ybuf = np.empty((ND, C, 224, WO), np.float32)
        for t in range(NT):
            i0 = STRIDE * t
            nro = 12 if t == NT - 1 else 16
            b0 = yr[t, 0, 0:ND * nro].reshape(ND, nro, C, WO)
            ybuf[:, :, i0:i0 + nro] = b0.transpose(0, 2, 1, 3)
            if t < NT - 1:
                b1 = yr[t, 1, 0:M1].reshape(ND, 14, C, WO)
                ybuf[:, :, i0 + 16:i0 + 30] = b1.transpose(0, 2, 1, 3)
        ys.append(ybuf[:, :, :HO, :].reshape(ND * C, HO, WO))
    y = np.stack(ys, axis=0)
    if _trace:
        return y, res
    return y
